# revision 47
# baseline (speedup 1.0000x reference)
"""CvT-style attention block (nn_Attention_38130719654007) on 8 Trainium2 cores.

Reference computation:
  - depthwise 3x3 conv + eval-mode BN on the 48x48 spatial tokens (cls token
    bypasses the conv) for each of q/k/v
  - linear projections Wq/Wk/Wv, 6-head attention over T=2305 with
    scale = C**-0.5, output projection Wo + bo.

Sharding: 8 cores = 4 batches x 2 query-halves. Each core computes full K/V
for its batch and attention for its half of the queries (1153 rows).

Device kernel (per core, all matmuls bf16, accumulation fp32):
  - host passes channel-transposed, halo-padded inputs (xT layouts, even and
    odd alignments) plus per-tap diagonal weight matrices
  - K/V depthwise conv runs on the TENSOR engine: 9 shifted diag-matmuls per
    channel chunk accumulated in PSUM (BN scale folded into the diagonals,
    BN shift folded into the PSUM->SBUF copy); Q conv runs on the vector
    engine in parallel.  Tiny border fixes repair the horizontal wrap.
  - conv blocks (512 tokens) are interleaved with the projection matmuls and
    the first attention pass so PE/ACT/DVE all stay busy from the start
  - Q^T,K^T produced channel-major [C,T]; V token-major with an appended
    ones column per head (row 64 of each head's O^T = softmax denominator)
  - S^T = K^T.T @ Q^T per head (K=64 matmuls, the two heads of a pair run in
    disjoint PE row groups); exp on the scalar engine straight out of PSUM
    (scale folded into the activation's free affine; exp args are in
    [-1.3, 1.3] so no max-subtraction is needed); O^T accumulates over the
    19 key chunks with the O matmuls trailing one chunk behind the exps
  - normalize by the ones-column sums, output projection with bias folded in
    as a K=1 matmul against a ones row.

Self-contained: accepts FULL inputs, returns the FULL output.
"""

import numpy as np

B, T, C, HEADS = 4, 2305, 384, 6
HW = 48
DH = C // HEADS  # 64
BN_EPS = 1e-5
NCORES = 8

NQ = 1153            # queries per core (incl. cls / pad)
XKV_COLS = 2448      # 51*48 : col 0 pad, 1..2400 = halo rows -1..48, rest 0
XQ_COLS = 1296       # 27*48 : col 0 pad, 1..1248 = 26 halo rows, rest 0
LB_LIST = [(0, 384), (384, 384), (768, 385)]  # query l-blocks
NTCH = 18            # spatial key chunks of 128 (2304 = 18*128)
KBLKS = [(0, 512), (512, 512), (1024, 512), (1536, 512), (2048, 256)]

_CACHE = {}

# tuning knobs (read at build time)
CFG = {"GRP": 1, "S_BUFS": 3, "PSE_BUFS": 2, "PT_BUFS": 10, "WARMUP": 0, "TRAIL": 3}


def _build_program():
    """Build + compile the per-core Bass program (cached)."""
    if "nc" in _CACHE:
        return _CACHE["nc"]

    import concourse.bass as bass
    import concourse.mybir as mybir
    import concourse.tile as tile
    from concourse import bacc

    f32 = mybir.dt.float32
    bf16 = mybir.dt.bfloat16
    AF = mybir.ActivationFunctionType
    OP = mybir.AluOpType

    nc = bacc.Bacc(
        "TRN2",
        target_bir_lowering=False,
        debug=False,
        enable_asserts=True,
        num_devices=NCORES,
    )

    # ---- DRAM I/O (per core) ----
    d_xkvT = nc.dram_tensor("xkvT", [C, XKV_COLS], bf16, kind="ExternalInput").ap()
    d_xqT = nc.dram_tensor("xqT", [C, XQ_COLS], bf16, kind="ExternalInput").ap()
    d_xclsT = nc.dram_tensor("xclsT", [C, 1], bf16, kind="ExternalInput").ap()
    # packed constants (partition-major, one DMA each):
    #   weights  [128, 3*384]: col block kc = W.T[kc chunk]
    #   ks       [128, 3*16]:  col block kc = conv scalars
    #   diag     [128, 27*128]: col block (kc*9+o) = diag(k'_eff[kc chunk, o])
    d_wq = nc.dram_tensor("wqT", [128, 3 * C], bf16, kind="ExternalInput").ap()
    d_wk = nc.dram_tensor("wkT", [128, 3 * C], bf16, kind="ExternalInput").ap()
    d_wv = nc.dram_tensor("wvT", [128, 3 * C], bf16, kind="ExternalInput").ap()
    d_wo = nc.dram_tensor("woT", [128, 3 * C], bf16, kind="ExternalInput").ap()
    d_kq = nc.dram_tensor("kq16", [128, 48], f32, kind="ExternalInput").ap()
    d_kk = nc.dram_tensor("kk16", [128, 48], f32, kind="ExternalInput").ap()
    d_kv = nc.dram_tensor("kv16", [128, 48], f32, kind="ExternalInput").ap()
    d_dgq = nc.dram_tensor("dgq", [128, 27 * 128], bf16, kind="ExternalInput").ap()
    d_dgk = nc.dram_tensor("dgk", [128, 27 * 128], bf16, kind="ExternalInput").ap()
    d_dgv = nc.dram_tensor("dgv", [128, 27 * 128], bf16, kind="ExternalInput").ap()
    d_bo = nc.dram_tensor("bo", [1, C], bf16, kind="ExternalInput").ap()
    d_out = nc.dram_tensor("out", [NQ, C], f32, kind="ExternalOutput").ap()

    SCALE = float(C) ** -0.5
    OFFS = [di * 48 + dj for di in range(3) for dj in range(3)]
    SB = CFG["S_BUFS"]

    with tile.TileContext(nc) as tc:
        with (
            tc.tile_pool(name="consts", bufs=1) as consts,
            tc.tile_pool(name="bigs", bufs=1) as bigs,
            tc.tile_pool(name="ptp", bufs=CFG["PT_BUFS"]) as ptp,
            tc.tile_pool(name="small", bufs=3) as small,
            tc.tile_pool(name="outp", bufs=3) as outp,
            tc.tile_pool(name="clsP", bufs=4) as clsP,
            tc.tile_pool(name="psA", bufs=1, space="PSUM") as psA,
        ):
            # ---- x loads first (q side, then diag/scalars, then kv) so the
            # DVE q-conv and the PE kv-conv can start as early as possible
            xqT, xkvT = [], []

            def load_x(lst, dram, cols, nm):
                for kc in range(3):
                    xt = bigs.tile([128, cols], bf16, name=f"{nm}{kc}",
                                   tag=f"{nm}{kc}")
                    nc.sync.dma_start(out=xt, in_=dram[kc * 128:(kc + 1) * 128, :])
                    lst.append(xt)

            # warm-up chain: keep the PE busy (and the HAM clock-gate
            # open) while the first DMAs land; no DMA dependencies.
            wones = consts.tile([1, 640], bf16, name="wones")
            nc.vector.memset(wones, 0.001)
            wps = psA.tile([128, 512], f32, tag="S0", bufs=SB, name="wps")
            for i in range(CFG["WARMUP"]):
                nc.tensor.matmul(wps, wones[:, 0:128], wones[:, 128:640],
                                 start=True, stop=True)

            dgt, kst, w_sb = {}, {}, {}
            for nm, dd in (("q", d_kq), ("k", d_kk), ("v", d_kv)):
                kt = consts.tile([128, 48], f32, name=f"ks{nm}", tag=f"ks{nm}")
                nc.sync.dma_start(out=kt, in_=dd)
                kst[nm] = kt
            ks_sb = {nm: [kst[nm][:, kc * 16:(kc + 1) * 16] for kc in range(3)]
                     for nm in ("q", "k", "v")}
            load_x(xqT, d_xqT, XQ_COLS, "xqT")
            dgq = consts.tile([128, 27 * 128], bf16, name="dgq", tag="dgq")
            nc.sync.dma_start(out=dgq, in_=d_dgq)
            dgt["q"] = dgq
            wq_t = consts.tile([128, 3 * C], bf16, name="wq_t", tag="wq_t")
            nc.sync.dma_start(out=wq_t, in_=d_wq)
            w_sb["q"] = [wq_t[:, kc * C:(kc + 1) * C] for kc in range(3)]
            dgk = consts.tile([128, 27 * 128], bf16, name="dgk", tag="dgk")
            nc.sync.dma_start(out=dgk, in_=d_dgk)
            dgt["k"] = dgk
            load_x(xkvT, d_xkvT, XKV_COLS, "xkvT")
            dgv = consts.tile([128, 27 * 128], bf16, name="dgv", tag="dgv")
            nc.sync.dma_start(out=dgv, in_=d_dgv)
            dgt["v"] = dgv
            dg_sb = {nm: [[dgt[nm][:, (kc * 9 + o) * 128:(kc * 9 + o + 1) * 128]
                           for o in range(9)] for kc in range(3)]
                     for nm in ("q", "k", "v")}


            for nm, dd in (("k", d_wk), ("v", d_wv), ("o", d_wo)):
                wt = consts.tile([128, 3 * C], bf16, name=f"w{nm}", tag=f"w{nm}")
                nc.sync.dma_start(out=wt, in_=dd)
                w_sb[nm] = [wt[:, kc * C:(kc + 1) * C] for kc in range(3)]
            xcls_sb = []
            for kc in range(3):
                xc = consts.tile([128, 1], bf16, name=f"xcls{kc}", tag=f"xcls{kc}")
                nc.sync.dma_start(out=xc, in_=d_xclsT[kc * 128:(kc + 1) * 128, :])
                xcls_sb.append(xc)
            bo_sb = consts.tile([1, C], bf16)
            nc.sync.dma_start(out=bo_sb, in_=d_bo)
            ones_sb = consts.tile([1, 128], bf16)
            nc.vector.memset(ones_sb, 1.0)

            def border_fixes(y, xT, ksc, b0, b1, eng):
                """Repair the horizontal wrap for output cols in [b0, b1)."""
                y3 = y.rearrange("p (i j) -> p i j", j=48)
                x3 = xT.rearrange("p (i j) -> p i j", j=48)
                # j=0 cols: t = i*48 in [b0,b1)
                i0, i1 = -(-b0 // 48), -(-b1 // 48)
                # j=47 cols: t = i*48+47 in [b0,b1)
                g0, g1 = -(-(b0 - 47) // 48), -(-(b1 - 47) // 48)
                for di in range(3):
                    if i1 > i0:
                        eng.scalar_tensor_tensor(
                            y3[:, i0:i1, 0:1],
                            x3[:, i0 + di:i1 + di, 0:1],
                            ksc[:, 9 + di:10 + di],
                            y3[:, i0:i1, 0:1], OP.mult, OP.add)
                    if g1 > g0:
                        eng.scalar_tensor_tensor(
                            y3[:, g0:g1, 47:48],
                            x3[:, g0 + di + 1:g1 + di + 1, 1:2],
                            ksc[:, 12 + di:13 + di],
                            y3[:, g0:g1, 47:48], OP.mult, OP.add)

            # taps ordered even-offsets first so work can start before the
            # odd-aligned x copy has arrived from HBM
            TAP_ORDER = [0, 2, 3, 5, 6, 8, 1, 4, 7]

            def conv_pe(y, xT, dgrow, ksc, b0, blen):
                """Depthwise conv on PE: 9 shifted diag-matmuls into PSUM,
                then a DVE copy (+BN shift) into SBUF and border fixes."""
                yp = psA.tile([128, blen], f32, tag="S0", bufs=SB, name="yp")
                for i, o in enumerate(TAP_ORDER):
                    nc.tensor.matmul(
                        yp, dgrow[o], xT[:, OFFS[o] + b0: OFFS[o] + b0 + blen],
                        start=(i == 0), stop=(i == 8))
                nc.vector.tensor_scalar(
                    y[:, b0:b0 + blen], yp, 1.0, ksc[:, 15:16],
                    OP.mult, OP.add)
                border_fixes(y, xT, ksc, b0, b0 + blen, nc.vector)

            # ---- persistent SBUF tensors ----
            y_q = [bigs.tile([128, 1152], bf16, name=f"y_q{kc}", tag=f"y_q{kc}")
                   for kc in range(3)]
            y_k = [bigs.tile([128, 2304], bf16, name=f"y_k{kc}", tag=f"y_k{kc}")
                   for kc in range(3)]
            y_v = [bigs.tile([128, 2304], bf16, name=f"y_v{kc}", tag=f"y_v{kc}")
                   for kc in range(3)]
            QT = [bigs.tile([128, NQ], bf16, name=f"QT{m}", tag=f"QT{m}")
                  for m in range(3)]
            KT = [bigs.tile([128, T], bf16, name=f"KT{m}", tag=f"KT{m}")
                  for m in range(3)]
            V = [bigs.tile([128, 390], bf16, name=f"V{tt}", tag=f"V{tt}")
                 for tt in range(NTCH)]
            Vcls = bigs.tile([1, 390], bf16)
            OnT = [bigs.tile([128, NQ], bf16, name=f"OnT{m}", tag=f"OnT{m}")
                   for m in range(3)]

            # ---- Q path (PE conv, emitted first: QT gates attention) ----
            for qb in range(3):
                for kc in range(3):
                    conv_pe(y_q[kc], xqT[kc], dg_sb["q"][kc], ks_sb["q"][kc],
                            qb * 384, 384)

            def kv_block(b):
                """PE conv + projections for token block b (cols b0..b0+bl)."""
                b0, bl = KBLKS[b]
                for kc in range(3):
                    conv_pe(y_k[kc], xkvT[kc], dg_sb["k"][kc], ks_sb["k"][kc],
                            b0, bl)
                for kc in range(3):
                    conv_pe(y_v[kc], xkvT[kc], dg_sb["v"][kc], ks_sb["v"][kc],
                            b0, bl)
                for m in range(3):
                    pk = psA.tile([128, bl], f32, tag="S1", bufs=SB, name="pk")
                    for kc in range(3):
                        nc.tensor.matmul(
                            pk, w_sb["k"][kc][:, m * 128:(m + 1) * 128],
                            y_k[kc][:, b0:b0 + bl],
                            start=(kc == 0), stop=(kc == 2))
                    nc.vector.tensor_copy(KT[m][:, b0:b0 + bl], pk)
                for tt in range(b0 // 128, (b0 + bl) // 128):
                    pv = psA.tile([128, C], f32, tag="S1", bufs=SB, name="pv")
                    for kc in range(3):
                        nc.tensor.matmul(
                            pv, y_v[kc][:, tt * 128:(tt + 1) * 128],
                            w_sb["v"][kc], start=(kc == 0), stop=(kc == 2))
                    v3 = V[tt].rearrange("p (h e) -> p h e", e=65)
                    nc.vector.tensor_copy(
                        v3[:, :, 0:64], pv.rearrange("p (h e) -> p h e", e=64))
                    nc.vector.memset(v3[:, :, 64:65], 1.0)

            # first K/V block, then the Q projection (q conv runs on DVE
            # meanwhile), then attention with the remaining K/V blocks
            # interleaved into the first (l-block, pair) chunk loop.
            kv_block(0)
            kv_block(1)

            # cls column of QT / KT
            for nm, dst, col in (("q", QT, 0), ("k", KT, 2304)):
                for m in range(3):
                    pc = psA.tile([128, 1], f32, tag="S0", bufs=SB, name="pc")
                    for kc in range(3):
                        nc.tensor.matmul(
                            pc, w_sb[nm][kc][:, m * 128:(m + 1) * 128],
                            xcls_sb[kc], start=(kc == 0), stop=(kc == 2))
                    nc.vector.tensor_copy(dst[m][:, col:col + 1], pc)
            # V cls row
            pvc = psA.tile([1, C], f32, tag="S1", bufs=SB, name="pvc")
            for kc in range(3):
                nc.tensor.matmul(pvc, xcls_sb[kc], w_sb["v"][kc],
                                 start=(kc == 0), stop=(kc == 2))
            vc3 = Vcls.rearrange("p (h e) -> p h e", e=65)
            nc.vector.tensor_copy(vc3[:, :, 0:64],
                                  pvc.rearrange("p (h e) -> p h e", e=64))
            nc.vector.memset(vc3[:, :, 64:65], 1.0)

            # QT spatial cols (write at col offset 1)
            for m in range(3):
                for tb0, tbl in ((0, 512), (512, 512), (1024, 128)):
                    pq = psA.tile([128, tbl], f32, tag="S0", bufs=SB, name="pq")
                    for kc in range(3):
                        nc.tensor.matmul(
                            pq, w_sb["q"][kc][:, m * 128:(m + 1) * 128],
                            y_q[kc][:, tb0:tb0 + tbl],
                            start=(kc == 0), stop=(kc == 2))
                    nc.vector.tensor_copy(QT[m][:, 1 + tb0:1 + tb0 + tbl], pq)

            # ---- attention ----
            first_iter = [True]
            kvb_done = [0]

            segs = [(l0, lb, pair) for (l0, lb) in LB_LIST
                    for pair in range(3)]
            OpBySeg = {}

            def emit_s(si, cchunk):
                """S matmul + exp for (segment, chunk); chunk NTCH = cls."""
                l0, lb, pair = segs[si]
                out = {}
                for hh in (2 * pair, 2 * pair + 1):
                    hb = 64 * (hh % 2)
                    if cchunk < NTCH:
                        t0 = cchunk * 128
                        S = psA.tile([128, lb], f32, tag=f"S{hh % 2}", bufs=SB, name="S")
                        nc.tensor.matmul(
                            S, KT[pair][hb:hb + 64, t0:t0 + 128],
                            QT[pair][hb:hb + 64, l0:l0 + lb])
                        PT = ptp.tile([128, lb], bf16, tag=f"PT{hh % 2}",
                                      name="PT")
                    else:
                        S = psA.tile([1, lb], f32, tag=f"S{hh % 2}", bufs=SB, name="Sc")
                        nc.tensor.matmul(
                            S, KT[pair][hb:hb + 64, 2304:2305],
                            QT[pair][hb:hb + 64, l0:l0 + lb])
                        PT = ptp.tile([1, lb], bf16, tag=f"PT{hh % 2}",
                                      name="Pc")
                    nc.scalar.activation(PT, S, AF.Exp, scale=SCALE)
                    out[hh] = PT
                return out

            def emit_o(si, cchunk, PTs):
                l0, lb, pair = segs[si]
                if cchunk == 0:
                    OpBySeg[si] = {
                        hh: psA.tile([65, lb], f32, tag=f"O{hh % 2}", bufs=1, name="Op")
                        for hh in (2 * pair, 2 * pair + 1)}
                Op = OpBySeg[si]
                for hh in (2 * pair, 2 * pair + 1):
                    if cchunk < NTCH:
                        nc.tensor.matmul(
                            Op[hh], V[cchunk][:, hh * 65:hh * 65 + 65],
                            PTs[hh], start=(cchunk == 0), stop=False)
                    else:
                        nc.tensor.matmul(
                            Op[hh], Vcls[:, hh * 65:hh * 65 + 65],
                            PTs[hh], start=False, stop=True)

            def emit_norm(si):
                # normalize rows 0..63 by row 64 (the ones-column sums).
                # reciprocal_approx_fast misreads PSUM on HW -> copy the
                # sums row to SBUF first.
                l0, lb, pair = segs[si]
                Op = OpBySeg.pop(si)
                for hh in (2 * pair, 2 * pair + 1):
                    # copy O out of PSUM right away so the bank frees fast
                    sums = small.tile([1, lb], f32, tag="sums", name="sums")
                    nc.vector.tensor_copy(sums, Op[hh][64:65, :])
                    rec = small.tile([1, lb], f32, tag="rec", name="rec")
                    nc.vector.reciprocal_approx_fast(out=rec, in_=sums)
                    rb = small.tile([64, lb], f32, tag="rb", name="rb")
                    nc.gpsimd.partition_broadcast(rb, rec)
                    po = 64 * (hh % 2)
                    nc.vector.tensor_mul(
                        OnT[hh // 2][po:po + 64, l0:l0 + lb],
                        Op[hh][0:64, :], rb)

            # one flat software-pipelined stream: O matmuls (and the segment
            # normalize) trail the S/exp stream by one chunk, so the PE
            # crosses segment boundaries without waiting on any exp.
            from collections import deque
            pend = deque()
            for si in range(len(segs)):
                for cchunk in range(NTCH + 1):
                    if si == 0 and cchunk in (4, 8, 12):
                        kv_block(cchunk // 4 + 1)
                        kvb_done[0] = cchunk // 4 + 1
                    pend.append((si, cchunk, emit_s(si, cchunk)))
                    if len(pend) > CFG["TRAIL"]:
                        psi, pc, pPT = pend.popleft()
                        emit_o(psi, pc, pPT)
                        if pc == NTCH:
                            emit_norm(psi)
            while pend:
                psi, pc, pPT = pend.popleft()
                emit_o(psi, pc, pPT)
                if pc == NTCH:
                    emit_norm(psi)

            # ---- output projection + bias, DMA out ----
            for t0 in range(0, NQ, 128):
                tl = min(128, NQ - t0)
                pout = psA.tile([128, C], f32, tag="S1", bufs=SB, name="pout")
                for kc in range(3):
                    nc.tensor.matmul(
                        pout[0:tl, :], OnT[kc][:, t0:t0 + tl], w_sb["o"][kc],
                        start=(kc == 0), stop=False)
                nc.tensor.matmul(pout[0:tl, :], ones_sb[:, 0:tl], bo_sb,
                                 start=False, stop=True)
                osb = outp.tile([128, C], f32, tag="osb", name="osb")
                nc.vector.tensor_copy(osb[0:tl, :], pout[0:tl, :])
                nc.sync.dma_start(out=d_out[t0:t0 + tl, :], in_=osb[0:tl, :])

    nc.compile()
    _CACHE["nc"] = nc
    return nc


def _prep_inputs(x, kq, kk, kv, gq, bq, mq, vq, gk, bk, mk, vk, gv, bv, mv, vv,
                 Wq, Wk, Wv, Wo, bo):
    """Host-side preprocessing -> per-core input maps."""
    import ml_dtypes
    bf = ml_dtypes.bfloat16

    def keff(kern, g, v):
        s = (np.asarray(g, np.float64) /
             np.sqrt(np.asarray(v, np.float64) + BN_EPS))
        return np.asarray(kern, np.float64)[:, 0].reshape(C, 9) * s[:, None]

    def ksc16(ke, b, m, g, v):
        s = (np.asarray(g, np.float64) /
             np.sqrt(np.asarray(v, np.float64) + BN_EPS))
        t = np.asarray(b, np.float64) - np.asarray(m, np.float64) * s
        out = np.zeros((C, 16), np.float32)
        out[:, 0:9] = ke
        out[:, 9:12] = -ke[:, [0, 3, 6]]
        out[:, 12:15] = -ke[:, [2, 5, 8]]
        out[:, 15] = t
        # pack [384, 16] -> [128, 3*16] (col block kc = chunk kc)
        return np.ascontiguousarray(
            out.reshape(3, 128, 16).transpose(1, 0, 2).reshape(128, 48))

    def diags(ke):
        # [128, 27*128]: col block (kc*9+o) = diag(ke[kc chunk, o])
        out = np.zeros((128, 27 * 128), np.float32)
        for kc in range(3):
            for o in range(9):
                blk = (kc * 9 + o) * 128
                out[np.arange(128), blk + np.arange(128)] = \
                    ke[kc * 128:(kc + 1) * 128, o]
        return out.astype(bf)

    def packw(W):
        # W.T [384, 384] -> [128, 3*384] (col block kc = W.T[kc chunk])
        wt = np.asarray(W, np.float32).T.reshape(3, 128, C)
        return np.ascontiguousarray(
            wt.transpose(1, 0, 2).reshape(128, 3 * C)).astype(bf)

    keq = keff(kq, gq, vq)
    kek = keff(kk, gk, vk)
    kev = keff(kv, gv, vv)

    common = {
        "wqT": packw(Wq),
        "wkT": packw(Wk),
        "wvT": packw(Wv),
        "woT": packw(Wo),
        "kq16": ksc16(keq, bq, mq, gq, vq),
        "kk16": ksc16(kek, bk, mk, gk, vk),
        "kv16": ksc16(kev, bv, mv, gv, vv),
        "dgq": diags(keq),
        "dgk": diags(kek),
        "dgv": diags(kev),
        "bo": np.asarray(bo, np.float32).reshape(1, C).astype(bf),
    }

    x = np.asarray(x, np.float32)
    in_maps = []
    for core in range(NCORES):
        b, half = core // 2, core % 2
        grid = x[b, 1:].reshape(HW, HW, C)
        # K/V input: halo rows -1..48 flattened at cols 1..2400
        xkv = np.zeros((XKV_COLS, C), np.float32)
        xkv[49:49 + 2304] = grid.reshape(2304, C)
        # Q input: halo rows r0-1..r0+24 at cols 1..1248
        r0 = 0 if half == 0 else 24
        xq = np.zeros((XQ_COLS, C), np.float32)
        g0, g1 = max(r0 - 1, 0), min(r0 + 25, HW)
        xq[1 + (g0 - (r0 - 1)) * HW: 1 + (g1 - (r0 - 1)) * HW] = \
            grid[g0:g1].reshape(-1, C)
        in_maps.append({
            "xkvT": np.ascontiguousarray(xkv.T).astype(bf),
            "xqT": np.ascontiguousarray(xq.T).astype(bf),
            "xclsT": np.ascontiguousarray(x[b, 0:1].T).astype(bf),
            **common,
        })
    return in_maps


def _run(in_maps, trace=False, **kw):
    from concourse import bass_utils
    nc = _build_program()
    return bass_utils.run_bass_kernel_spmd(
        nc, in_maps, core_ids=list(range(len(in_maps))), trace=trace, **kw)


def kernel(x, kq, kk, kv, gq, bq, mq, vq, gk, bk, mk, vk, gv, bv, mv, vv,
           Wq, Wk, Wv, Wo, bo, h, w):
    in_maps = _prep_inputs(x, kq, kk, kv, gq, bq, mq, vq, gk, bk, mk, vk,
                           gv, bv, mv, vv, Wq, Wk, Wv, Wo, bo)
    res = _run(in_maps)
    out = np.empty((B, T, C), np.float32)
    for core in range(NCORES):
        b, half = core // 2, core % 2
        o = res.results[core]["out"]
        if half == 0:
            out[b, 0:1153] = o
        else:
            out[b, 1153:2305] = o[1:1153]
    return out


# revision 48
# speedup vs baseline: 1.1656x; 1.1656x over previous
"""CvT-style attention block (nn_Attention_38130719654007) on 8 Trainium2 cores.

Reference computation:
  - depthwise 3x3 conv + eval-mode BN on the 48x48 spatial tokens (cls token
    bypasses the conv) for each of q/k/v
  - linear projections Wq/Wk/Wv, 6-head attention over T=2305 with
    scale = C**-0.5, output projection Wo + bo.

Sharding: 8 cores = 4 batches x 2 query-halves. Each core computes full K/V
for its batch and attention for its half of the queries (1153 rows).

Device kernel (per core, all matmuls bf16, accumulation fp32):
  - host passes channel-transposed, halo-padded inputs (xT layouts, even and
    odd alignments) plus per-tap diagonal weight matrices
  - K/V depthwise conv runs on the TENSOR engine: 9 shifted diag-matmuls per
    channel chunk accumulated in PSUM (BN scale folded into the diagonals,
    BN shift folded into the PSUM->SBUF copy); Q conv runs on the vector
    engine in parallel.  Tiny border fixes repair the horizontal wrap.
  - conv blocks (512 tokens) are interleaved with the projection matmuls and
    the first attention pass so PE/ACT/DVE all stay busy from the start
  - Q^T,K^T produced channel-major [C,T]; V token-major with an appended
    ones column per head (row 64 of each head's O^T = softmax denominator)
  - S^T = K^T.T @ Q^T per head (K=64 matmuls, the two heads of a pair run in
    disjoint PE row groups); exp on the scalar engine straight out of PSUM
    (scale folded into the activation's free affine; exp args are in
    [-1.3, 1.3] so no max-subtraction is needed); O^T accumulates over the
    19 key chunks with the O matmuls trailing one chunk behind the exps
  - normalize by the ones-column sums, output projection with bias folded in
    as a K=1 matmul against a ones row.

Self-contained: accepts FULL inputs, returns the FULL output.
"""

import numpy as np

B, T, C, HEADS = 4, 2305, 384, 6
HW = 48
DH = C // HEADS  # 64
BN_EPS = 1e-5
NCORES = 8

NQ = 1153            # queries per core (incl. cls / pad)
XKV_COLS = 2448      # 51*48 : col 0 pad, 1..2400 = halo rows -1..48, rest 0
XQ_COLS = 1296       # 27*48 : col 0 pad, 1..1248 = 26 halo rows, rest 0
LB_LIST = [(0, 512), (512, 512), (1024, 129)]  # query l-blocks
NTCH = 18            # spatial key chunks of 128 (2304 = 18*128)
KBLKS = [(0, 512), (512, 512), (1024, 512), (1536, 512), (2048, 256)]

_CACHE = {}

# tuning knobs (read at build time)
CFG = {"GRP": 1, "S_BUFS": 3, "PSE_BUFS": 2, "PT_BUFS": 10, "WARMUP": 0, "TRAIL": 3}


def _build_program():
    """Build + compile the per-core Bass program (cached)."""
    if "nc" in _CACHE:
        return _CACHE["nc"]

    import concourse.bass as bass
    import concourse.mybir as mybir
    import concourse.tile as tile
    from concourse import bacc

    f32 = mybir.dt.float32
    bf16 = mybir.dt.bfloat16
    AF = mybir.ActivationFunctionType
    OP = mybir.AluOpType

    nc = bacc.Bacc(
        "TRN2",
        target_bir_lowering=False,
        debug=False,
        enable_asserts=True,
        num_devices=NCORES,
    )

    # ---- DRAM I/O (per core) ----
    d_xkvT = nc.dram_tensor("xkvT", [C, XKV_COLS], bf16, kind="ExternalInput").ap()
    d_xqT = nc.dram_tensor("xqT", [C, XQ_COLS], bf16, kind="ExternalInput").ap()
    d_xqTo = nc.dram_tensor("xqTo", [C, XQ_COLS], bf16, kind="ExternalInput").ap()
    d_xclsT = nc.dram_tensor("xclsT", [C, 1], bf16, kind="ExternalInput").ap()
    # packed constants (partition-major, one DMA each):
    #   weights  [128, 3*384]: col block kc = W.T[kc chunk]
    #   ks       [128, 3*16]:  col block kc = conv scalars
    #   diag     [128, 27*128]: col block (kc*9+o) = diag(k'_eff[kc chunk, o])
    d_wq = nc.dram_tensor("wqT", [128, 3 * C], bf16, kind="ExternalInput").ap()
    d_wk = nc.dram_tensor("wkT", [128, 3 * C], bf16, kind="ExternalInput").ap()
    d_wv = nc.dram_tensor("wvT", [128, 3 * C], bf16, kind="ExternalInput").ap()
    d_wo = nc.dram_tensor("woT", [128, 3 * C], bf16, kind="ExternalInput").ap()
    d_kq = nc.dram_tensor("kq16", [128, 48], f32, kind="ExternalInput").ap()
    d_kk = nc.dram_tensor("kk16", [128, 48], f32, kind="ExternalInput").ap()
    d_kv = nc.dram_tensor("kv16", [128, 48], f32, kind="ExternalInput").ap()
    d_dgk = nc.dram_tensor("dgk", [128, 27 * 128], bf16, kind="ExternalInput").ap()
    d_dgv = nc.dram_tensor("dgv", [128, 27 * 128], bf16, kind="ExternalInput").ap()
    d_bo = nc.dram_tensor("bo", [1, C], bf16, kind="ExternalInput").ap()
    d_out = nc.dram_tensor("out", [NQ, C], f32, kind="ExternalOutput").ap()

    SCALE = float(C) ** -0.5
    OFFS = [di * 48 + dj for di in range(3) for dj in range(3)]
    SB = CFG["S_BUFS"]

    with tile.TileContext(nc) as tc:
        with (
            tc.tile_pool(name="consts", bufs=1) as consts,
            tc.tile_pool(name="bigs", bufs=1) as bigs,
            tc.tile_pool(name="ptp", bufs=CFG["PT_BUFS"]) as ptp,
            tc.tile_pool(name="small", bufs=3) as small,
            tc.tile_pool(name="outp", bufs=3) as outp,
            tc.tile_pool(name="clsP", bufs=4) as clsP,
            tc.tile_pool(name="psA", bufs=1, space="PSUM") as psA,
        ):
            # ---- x loads first (q side, then diag/scalars, then kv) so the
            # DVE q-conv and the PE kv-conv can start as early as possible
            xqT, xqTo, xkvT = [], [], []

            def load_x(lst, dram, cols, nm):
                for kc in range(3):
                    xt = bigs.tile([128, cols], bf16, name=f"{nm}{kc}",
                                   tag=f"{nm}{kc}")
                    nc.sync.dma_start(out=xt, in_=dram[kc * 128:(kc + 1) * 128, :])
                    lst.append(xt)

            # warm-up chain: keep the PE busy (and the HAM clock-gate
            # open) while the first DMAs land; no DMA dependencies.
            wones = consts.tile([1, 640], bf16, name="wones")
            nc.vector.memset(wones, 0.001)
            wps = psA.tile([128, 512], f32, tag="S0", bufs=SB, name="wps")
            for i in range(CFG["WARMUP"]):
                nc.tensor.matmul(wps, wones[:, 0:128], wones[:, 128:640],
                                 start=True, stop=True)

            dgt, kst, w_sb = {}, {}, {}
            for nm, dd in (("q", d_kq), ("k", d_kk), ("v", d_kv)):
                kt = consts.tile([128, 48], f32, name=f"ks{nm}", tag=f"ks{nm}")
                nc.sync.dma_start(out=kt, in_=dd)
                kst[nm] = kt
            ks_sb = {nm: [kst[nm][:, kc * 16:(kc + 1) * 16] for kc in range(3)]
                     for nm in ("q", "k", "v")}
            dgk = consts.tile([128, 27 * 128], bf16, name="dgk", tag="dgk")
            nc.sync.dma_start(out=dgk, in_=d_dgk)
            dgt["k"] = dgk
            load_x(xkvT, d_xkvT, XKV_COLS, "xkvT")
            load_x(xqT, d_xqT, XQ_COLS, "xqT")
            load_x(xqTo, d_xqTo, XQ_COLS, "xqTo")
            dgv = consts.tile([128, 27 * 128], bf16, name="dgv", tag="dgv")
            nc.sync.dma_start(out=dgv, in_=d_dgv)
            dgt["v"] = dgv
            dg_sb = {nm: [[dgt[nm][:, (kc * 9 + o) * 128:(kc * 9 + o + 1) * 128]
                           for o in range(9)] for kc in range(3)]
                     for nm in ("k", "v")}


            for nm, dd in (("q", d_wq), ("k", d_wk), ("v", d_wv), ("o", d_wo)):
                wt = consts.tile([128, 3 * C], bf16, name=f"w{nm}", tag=f"w{nm}")
                nc.sync.dma_start(out=wt, in_=dd)
                w_sb[nm] = [wt[:, kc * C:(kc + 1) * C] for kc in range(3)]
            xcls_sb = []
            for kc in range(3):
                xc = consts.tile([128, 1], bf16, name=f"xcls{kc}", tag=f"xcls{kc}")
                nc.sync.dma_start(out=xc, in_=d_xclsT[kc * 128:(kc + 1) * 128, :])
                xcls_sb.append(xc)
            bo_sb = consts.tile([1, C], bf16)
            nc.sync.dma_start(out=bo_sb, in_=d_bo)
            ones_sb = consts.tile([1, 128], bf16)
            nc.vector.memset(ones_sb, 1.0)

            def border_fixes(y, xT, ksc, b0, b1, eng):
                """Repair the horizontal wrap for output cols in [b0, b1)."""
                y3 = y.rearrange("p (i j) -> p i j", j=48)
                x3 = xT.rearrange("p (i j) -> p i j", j=48)
                # j=0 cols: t = i*48 in [b0,b1)
                i0, i1 = -(-b0 // 48), -(-b1 // 48)
                # j=47 cols: t = i*48+47 in [b0,b1)
                g0, g1 = -(-(b0 - 47) // 48), -(-(b1 - 47) // 48)
                for di in range(3):
                    if i1 > i0:
                        eng.scalar_tensor_tensor(
                            y3[:, i0:i1, 0:1],
                            x3[:, i0 + di:i1 + di, 0:1],
                            ksc[:, 9 + di:10 + di],
                            y3[:, i0:i1, 0:1], OP.mult, OP.add)
                    if g1 > g0:
                        eng.scalar_tensor_tensor(
                            y3[:, g0:g1, 47:48],
                            x3[:, g0 + di + 1:g1 + di + 1, 1:2],
                            ksc[:, 12 + di:13 + di],
                            y3[:, g0:g1, 47:48], OP.mult, OP.add)

            # taps ordered even-offsets first so work can start before the
            # odd-aligned x copy has arrived from HBM
            TAP_ORDER = [0, 2, 3, 5, 6, 8, 1, 4, 7]

            def conv_dve(y, xT, xTo, ksc, b0, blen):
                """Depthwise conv on DVE: 9 scalar_tensor_tensor taps."""
                yb = y[:, b0:b0 + blen]
                nc.vector.tensor_scalar(
                    yb, xT[:, OFFS[0] + b0: OFFS[0] + b0 + blen],
                    ksc[:, 0:1], ksc[:, 15:16], OP.mult, OP.add)
                for o in TAP_ORDER[1:]:
                    off = OFFS[o]
                    src_ = xT if off % 2 == 0 else xTo
                    off = off if off % 2 == 0 else off - 1
                    nc.vector.scalar_tensor_tensor(
                        yb, src_[:, off + b0: off + b0 + blen],
                        ksc[:, o:o + 1], yb, OP.mult, OP.add)
                border_fixes(y, xT, ksc, b0, b0 + blen, nc.vector)

            def conv_pe(y, xT, dgrow, ksc, b0, blen):
                """Depthwise conv on PE: 9 shifted diag-matmuls into PSUM,
                then a DVE copy (+BN shift) into SBUF and border fixes."""
                yp = psA.tile([128, blen], f32, tag="S0", bufs=SB, name="yp")
                for i, o in enumerate(TAP_ORDER):
                    nc.tensor.matmul(
                        yp, dgrow[o], xT[:, OFFS[o] + b0: OFFS[o] + b0 + blen],
                        start=(i == 0), stop=(i == 8))
                nc.vector.tensor_scalar(
                    y[:, b0:b0 + blen], yp, 1.0, ksc[:, 15:16],
                    OP.mult, OP.add)
                border_fixes(y, xT, ksc, b0, b0 + blen, nc.vector)

            # ---- persistent SBUF tensors ----
            y_q = [bigs.tile([128, 1152], bf16, name=f"y_q{kc}", tag=f"y_q{kc}")
                   for kc in range(3)]
            y_k = [bigs.tile([128, 2304], bf16, name=f"y_k{kc}", tag=f"y_k{kc}")
                   for kc in range(3)]
            y_v = [bigs.tile([128, 2304], bf16, name=f"y_v{kc}", tag=f"y_v{kc}")
                   for kc in range(3)]
            QT = [bigs.tile([128, NQ], bf16, name=f"QT{m}", tag=f"QT{m}")
                  for m in range(3)]
            KT = [bigs.tile([128, T], bf16, name=f"KT{m}", tag=f"KT{m}")
                  for m in range(3)]
            V = [bigs.tile([128, 390], bf16, name=f"V{tt}", tag=f"V{tt}")
                 for tt in range(NTCH)]
            Vcls = bigs.tile([1, 390], bf16)
            OnT = [bigs.tile([128, NQ], bf16, name=f"OnT{m}", tag=f"OnT{m}")
                   for m in range(3)]

            # ---- Q path on DVE (runs while PE does K/V conv) ----
            for kc in range(3):
                conv_dve(y_q[kc], xqT[kc], xqTo[kc], ks_sb["q"][kc], 0, 1152)

            def kv_block(b):
                """PE conv + projections for token block b (cols b0..b0+bl)."""
                b0, bl = KBLKS[b]
                for kc in range(3):
                    conv_pe(y_k[kc], xkvT[kc], dg_sb["k"][kc], ks_sb["k"][kc],
                            b0, bl)
                for kc in range(3):
                    conv_pe(y_v[kc], xkvT[kc], dg_sb["v"][kc], ks_sb["v"][kc],
                            b0, bl)
                for m in range(3):
                    pk = psA.tile([128, bl], f32, tag="S1", bufs=SB, name="pk")
                    for kc in range(3):
                        nc.tensor.matmul(
                            pk, w_sb["k"][kc][:, m * 128:(m + 1) * 128],
                            y_k[kc][:, b0:b0 + bl],
                            start=(kc == 0), stop=(kc == 2))
                    nc.vector.tensor_copy(KT[m][:, b0:b0 + bl], pk)
                for tt in range(b0 // 128, (b0 + bl) // 128):
                    pv = psA.tile([128, C], f32, tag="S1", bufs=SB, name="pv")
                    for kc in range(3):
                        nc.tensor.matmul(
                            pv, y_v[kc][:, tt * 128:(tt + 1) * 128],
                            w_sb["v"][kc], start=(kc == 0), stop=(kc == 2))
                    v3 = V[tt].rearrange("p (h e) -> p h e", e=65)
                    nc.vector.tensor_copy(
                        v3[:, :, 0:64], pv.rearrange("p (h e) -> p h e", e=64))
                    nc.vector.memset(v3[:, :, 64:65], 1.0)

            # first K/V block, then the Q projection (q conv runs on DVE
            # meanwhile), then attention with the remaining K/V blocks
            # interleaved into the first (l-block, pair) chunk loop.
            kv_block(0)

            # cls column of QT / KT
            for nm, dst, col in (("q", QT, 0), ("k", KT, 2304)):
                for m in range(3):
                    pc = psA.tile([128, 1], f32, tag="S0", bufs=SB, name="pc")
                    for kc in range(3):
                        nc.tensor.matmul(
                            pc, w_sb[nm][kc][:, m * 128:(m + 1) * 128],
                            xcls_sb[kc], start=(kc == 0), stop=(kc == 2))
                    nc.vector.tensor_copy(dst[m][:, col:col + 1], pc)
            # V cls row
            pvc = psA.tile([1, C], f32, tag="S1", bufs=SB, name="pvc")
            for kc in range(3):
                nc.tensor.matmul(pvc, xcls_sb[kc], w_sb["v"][kc],
                                 start=(kc == 0), stop=(kc == 2))
            vc3 = Vcls.rearrange("p (h e) -> p h e", e=65)
            nc.vector.tensor_copy(vc3[:, :, 0:64],
                                  pvc.rearrange("p (h e) -> p h e", e=64))
            nc.vector.memset(vc3[:, :, 64:65], 1.0)

            # QT spatial cols (write at col offset 1)
            for m in range(3):
                for tb0, tbl in ((0, 512), (512, 512), (1024, 128)):
                    pq = psA.tile([128, tbl], f32, tag="S0", bufs=SB, name="pq")
                    for kc in range(3):
                        nc.tensor.matmul(
                            pq, w_sb["q"][kc][:, m * 128:(m + 1) * 128],
                            y_q[kc][:, tb0:tb0 + tbl],
                            start=(kc == 0), stop=(kc == 2))
                    nc.vector.tensor_copy(QT[m][:, 1 + tb0:1 + tb0 + tbl], pq)

            # ---- attention ----
            first_iter = [True]
            kvb_done = [0]

            segs = [(l0, lb, pair) for (l0, lb) in LB_LIST
                    for pair in range(3)]
            OpBySeg = {}

            def emit_s(si, cchunk):
                """S matmul + exp for (segment, chunk); chunk NTCH = cls."""
                l0, lb, pair = segs[si]
                out = {}
                for hh in (2 * pair, 2 * pair + 1):
                    hb = 64 * (hh % 2)
                    if cchunk < NTCH:
                        t0 = cchunk * 128
                        S = psA.tile([128, lb], f32, tag=f"S{hh % 2}", bufs=SB, name="S")
                        nc.tensor.matmul(
                            S, KT[pair][hb:hb + 64, t0:t0 + 128],
                            QT[pair][hb:hb + 64, l0:l0 + lb])
                        PT = ptp.tile([128, lb], bf16, tag=f"PT{hh % 2}",
                                      name="PT")
                    else:
                        S = psA.tile([1, lb], f32, tag=f"S{hh % 2}", bufs=SB, name="Sc")
                        nc.tensor.matmul(
                            S, KT[pair][hb:hb + 64, 2304:2305],
                            QT[pair][hb:hb + 64, l0:l0 + lb])
                        PT = ptp.tile([1, lb], bf16, tag=f"PT{hh % 2}",
                                      name="Pc")
                    nc.scalar.activation(PT, S, AF.Exp, scale=SCALE)
                    out[hh] = PT
                return out

            def emit_o(si, cchunk, PTs):
                l0, lb, pair = segs[si]
                if cchunk == 0:
                    OpBySeg[si] = {
                        hh: psA.tile([65, lb], f32, tag=f"O{hh % 2}", bufs=1, name="Op")
                        for hh in (2 * pair, 2 * pair + 1)}
                Op = OpBySeg[si]
                for hh in (2 * pair, 2 * pair + 1):
                    if cchunk < NTCH:
                        nc.tensor.matmul(
                            Op[hh], V[cchunk][:, hh * 65:hh * 65 + 65],
                            PTs[hh], start=(cchunk == 0), stop=False)
                    else:
                        nc.tensor.matmul(
                            Op[hh], Vcls[:, hh * 65:hh * 65 + 65],
                            PTs[hh], start=False, stop=True)

            def emit_norm(si):
                # normalize rows 0..63 by row 64 (the ones-column sums).
                # reciprocal_approx_fast misreads PSUM on HW -> copy the
                # sums row to SBUF first.
                l0, lb, pair = segs[si]
                Op = OpBySeg.pop(si)
                for hh in (2 * pair, 2 * pair + 1):
                    # copy O out of PSUM right away so the bank frees fast
                    sums = small.tile([1, lb], f32, tag="sums", name="sums")
                    nc.vector.tensor_copy(sums, Op[hh][64:65, :])
                    rec = small.tile([1, lb], f32, tag="rec", name="rec")
                    nc.vector.reciprocal_approx_fast(out=rec, in_=sums)
                    rb = small.tile([64, lb], f32, tag="rb", name="rb")
                    nc.gpsimd.partition_broadcast(rb, rec)
                    po = 64 * (hh % 2)
                    nc.vector.tensor_mul(
                        OnT[hh // 2][po:po + 64, l0:l0 + lb],
                        Op[hh][0:64, :], rb)

            # one flat software-pipelined stream: O matmuls (and the segment
            # normalize) trail the S/exp stream by one chunk, so the PE
            # crosses segment boundaries without waiting on any exp.
            from collections import deque
            pend = deque()
            for si in range(len(segs)):
                for cchunk in range(NTCH + 1):
                    if si == 0 and cchunk in (4, 8, 12, 16):
                        kv_block(cchunk // 4)
                        kvb_done[0] = cchunk // 4
                    pend.append((si, cchunk, emit_s(si, cchunk)))
                    if len(pend) > CFG["TRAIL"]:
                        psi, pc, pPT = pend.popleft()
                        emit_o(psi, pc, pPT)
                        if pc == NTCH:
                            emit_norm(psi)
            while pend:
                psi, pc, pPT = pend.popleft()
                emit_o(psi, pc, pPT)
                if pc == NTCH:
                    emit_norm(psi)

            # ---- output projection + bias, DMA out ----
            for t0 in range(0, NQ, 128):
                tl = min(128, NQ - t0)
                pout = psA.tile([128, C], f32, tag="S1", bufs=SB, name="pout")
                for kc in range(3):
                    nc.tensor.matmul(
                        pout[0:tl, :], OnT[kc][:, t0:t0 + tl], w_sb["o"][kc],
                        start=(kc == 0), stop=False)
                nc.tensor.matmul(pout[0:tl, :], ones_sb[:, 0:tl], bo_sb,
                                 start=False, stop=True)
                osb = outp.tile([128, C], f32, tag="osb", name="osb")
                nc.vector.tensor_copy(osb[0:tl, :], pout[0:tl, :])
                nc.sync.dma_start(out=d_out[t0:t0 + tl, :], in_=osb[0:tl, :])

    nc.compile()
    _CACHE["nc"] = nc
    return nc


def _prep_inputs(x, kq, kk, kv, gq, bq, mq, vq, gk, bk, mk, vk, gv, bv, mv, vv,
                 Wq, Wk, Wv, Wo, bo):
    """Host-side preprocessing -> per-core input maps."""
    import ml_dtypes
    bf = ml_dtypes.bfloat16

    def keff(kern, g, v):
        s = (np.asarray(g, np.float64) /
             np.sqrt(np.asarray(v, np.float64) + BN_EPS))
        return np.asarray(kern, np.float64)[:, 0].reshape(C, 9) * s[:, None]

    def ksc16(ke, b, m, g, v):
        s = (np.asarray(g, np.float64) /
             np.sqrt(np.asarray(v, np.float64) + BN_EPS))
        t = np.asarray(b, np.float64) - np.asarray(m, np.float64) * s
        out = np.zeros((C, 16), np.float32)
        out[:, 0:9] = ke
        out[:, 9:12] = -ke[:, [0, 3, 6]]
        out[:, 12:15] = -ke[:, [2, 5, 8]]
        out[:, 15] = t
        # pack [384, 16] -> [128, 3*16] (col block kc = chunk kc)
        return np.ascontiguousarray(
            out.reshape(3, 128, 16).transpose(1, 0, 2).reshape(128, 48))

    def diags(ke):
        # [128, 27*128]: col block (kc*9+o) = diag(ke[kc chunk, o])
        out = np.zeros((128, 27 * 128), np.float32)
        for kc in range(3):
            for o in range(9):
                blk = (kc * 9 + o) * 128
                out[np.arange(128), blk + np.arange(128)] = \
                    ke[kc * 128:(kc + 1) * 128, o]
        return out.astype(bf)

    def packw(W):
        # W.T [384, 384] -> [128, 3*384] (col block kc = W.T[kc chunk])
        wt = np.asarray(W, np.float32).T.reshape(3, 128, C)
        return np.ascontiguousarray(
            wt.transpose(1, 0, 2).reshape(128, 3 * C)).astype(bf)

    keq = keff(kq, gq, vq)
    kek = keff(kk, gk, vk)
    kev = keff(kv, gv, vv)

    common = {
        "wqT": packw(Wq),
        "wkT": packw(Wk),
        "wvT": packw(Wv),
        "woT": packw(Wo),
        "kq16": ksc16(keq, bq, mq, gq, vq),
        "kk16": ksc16(kek, bk, mk, gk, vk),
        "kv16": ksc16(kev, bv, mv, gv, vv),
        "dgk": diags(kek),
        "dgv": diags(kev),
        "bo": np.asarray(bo, np.float32).reshape(1, C).astype(bf),
    }

    x = np.asarray(x, np.float32)
    in_maps = []
    for core in range(NCORES):
        b, half = core // 2, core % 2
        grid = x[b, 1:].reshape(HW, HW, C)
        # K/V input: halo rows -1..48 flattened at cols 1..2400
        xkv = np.zeros((XKV_COLS, C), np.float32)
        xkv[49:49 + 2304] = grid.reshape(2304, C)
        # Q input: halo rows r0-1..r0+24 at cols 1..1248
        r0 = 0 if half == 0 else 24
        xq = np.zeros((XQ_COLS, C), np.float32)
        g0, g1 = max(r0 - 1, 0), min(r0 + 25, HW)
        xq[1 + (g0 - (r0 - 1)) * HW: 1 + (g1 - (r0 - 1)) * HW] = \
            grid[g0:g1].reshape(-1, C)
        xq_o = np.zeros_like(xq)
        xq_o[:-1] = xq[1:]
        in_maps.append({
            "xkvT": np.ascontiguousarray(xkv.T).astype(bf),
            "xqT": np.ascontiguousarray(xq.T).astype(bf),
            "xqTo": np.ascontiguousarray(xq_o.T).astype(bf),
            "xclsT": np.ascontiguousarray(x[b, 0:1].T).astype(bf),
            **common,
        })
    return in_maps


def _run(in_maps, trace=False, **kw):
    from concourse import bass_utils
    nc = _build_program()
    return bass_utils.run_bass_kernel_spmd(
        nc, in_maps, core_ids=list(range(len(in_maps))), trace=trace, **kw)


def kernel(x, kq, kk, kv, gq, bq, mq, vq, gk, bk, mk, vk, gv, bv, mv, vv,
           Wq, Wk, Wv, Wo, bo, h, w):
    in_maps = _prep_inputs(x, kq, kk, kv, gq, bq, mq, vq, gk, bk, mk, vk,
                           gv, bv, mv, vv, Wq, Wk, Wv, Wo, bo)
    res = _run(in_maps)
    out = np.empty((B, T, C), np.float32)
    for core in range(NCORES):
        b, half = core // 2, core % 2
        o = res.results[core]["out"]
        if half == 0:
            out[b, 0:1153] = o
        else:
            out[b, 1153:2305] = o[1:1153]
    return out
